# revision 14
# baseline (speedup 1.0000x reference)
"""Trainium2 Bass kernel for a 2-layer GRU LM step (nn_GRU_83519934038617).

Model (fp32 reference):
  x = emb[inputs]                                  [S=64, B=32, E=512]
  2 GRU layers (H=512), logits = outs @ Wout.T + bout   [S, B, V=32000]
  returns (logits, stacked final hidden [2, B, H])

Strategy (8 cores, no collectives):
  - GRU recurrence replicated on every core (it is weight-streaming bound,
    so batch-sharding would not make it faster); vocab dim of the output
    projection sharded 8 ways (V_local = 4000) since logits dominate memory.
  - Per step, gates are computed with 4 concurrent column-group matmuls
    (tile_position col tiling, bf16) so the gate pre-activations land in
    PSUM as [128 partitions = (chunk j, batch b), 512 = r|z|hh|xh], which
    makes all elementwise work full-width [128, *] ops.
  - x-projections (xg1) are batched into one big GEMM up front; their
    per-step contribution enters the gate PSUM through an identity-
    stationary matmul (no DVE adds). Layer-2's input projection is folded
    into the per-step matmul as extra contraction tiles over hT1(t).
  - h is kept in two layouts: [(j,b), h'] for elementwise, and a bf16
    transpose [h', (kc,b)] (produced by one PE transpose per step) that
    feeds the next step's stationaries and accumulates into outsT for the
    final fp32r output GEMM.
"""
import numpy as np
import ml_dtypes
from contextlib import ExitStack

import concourse.tile as tile
from concourse import bacc, mybir
from concourse.bass_utils import run_bass_kernel_spmd

F32 = mybir.dt.float32
F32R = mybir.dt.float32r
BF16 = mybir.dt.bfloat16
BF16NP = ml_dtypes.bfloat16

S, B, E, H, V, L = 64, 32, 512, 512, 32000, 2
NCORES = 8
VL = V // NCORES          # 4000
T = S * B                 # 2048 tokens
NJ = 4                    # column groups (h-chunks of 128)
GW = 384                  # r|z|h group width in the 1536 weight layout
PW = 512                  # psum width per step: r|z|hh|xh
VC = 500                  # vocab chunk for the output GEMM (8 chunks)
NVC = VL // VC            # 8

SIG = mybir.ActivationFunctionType.Sigmoid
TANH = mybir.ActivationFunctionType.Tanh
COPY = mybir.ActivationFunctionType.Copy


def _perm1536(order):
    # new col j*384 + s*128 + hp  <-  old col order[s]*512 + 128*j + hp
    # order maps section s (within a 384-wide group) to gate index
    # (0=r, 1=z, 2=h in the concatenated [r|z|h] layout).
    perm = np.empty(1536, np.int64)
    for j in range(4):
        for s, g in enumerate(order):
            for hp in range(128):
                perm[j * 384 + s * 128 + hp] = g * 512 + 128 * j + hp
    return perm


def _build():
    nc = bacc.Bacc("TRN2", target_bir_lowering=False, debug=False,
                   num_devices=NCORES)

    def din(name, shape, dt):
        return nc.dram_tensor(name, shape, dt, kind="ExternalInput").ap()

    xt_d = din("xt", [E, T], BF16)
    wi1_d = din("wi1", [E, 1536], BF16)
    wh1_d = din("wh1", [H, 1536], BF16)
    wi2_d = din("wi2", [H, 1536], BF16)
    wh2_d = din("wh2", [H, 1536], BF16)
    b1_d = din("b1", [128, 1536], F32)   # broadcast along partitions (DVE add)
    b2_d = din("b2", [1, 1536], BF16)
    h0p_d = din("h0p", [L, 128, 128], F32)
    h0t_d = din("h0t", [L, 128, 128], BF16)
    eyestk_d = din("eyestk", [128, 32], BF16)
    eye128_d = din("eye128", [128, 128], F32)
    ones32_d = din("ones32", [1, 32], BF16)
    ones128b_d = din("ones128b", [1, 128], BF16)
    ones128r_d = din("ones128r", [1, 128], F32R)
    wout_d = din("wout", [H, VL], F32R)
    bout_d = din("bout", [128, VL], F32)  # broadcast along partitions (DVE add)

    logits_d = nc.dram_tensor("logits", [T, VL], F32, kind="ExternalOutput").ap()
    hfin_d = nc.dram_tensor("hfin", [L, 128, 128], F32, kind="ExternalOutput").ap()

    with tile.TileContext(nc) as tc, ExitStack() as ctx:
        persist = ctx.enter_context(tc.tile_pool(name="persist", bufs=1))
        xt_pool = ctx.enter_context(tc.tile_pool(name="xt", bufs=2))
        wout_pool = ctx.enter_context(tc.tile_pool(name="wout", bufs=8))
        lg_pool = ctx.enter_context(tc.tile_pool(name="lg", bufs=3))
        srz_pool = ctx.enter_context(tc.tile_pool(name="srz", bufs=4))
        tmp_pool = ctx.enter_context(tc.tile_pool(name="tmp", bufs=8))
        h_pool = ctx.enter_context(tc.tile_pool(name="hp", bufs=6))
        ht_pool = ctx.enter_context(tc.tile_pool(name="htp", bufs=6))
        ps_big = ctx.enter_context(tc.tile_pool(name="psbig", bufs=2, space="PSUM"))
        ps_rec = ctx.enter_context(tc.tile_pool(name="psrec", bufs=3, space="PSUM"))
        ps_tr = ctx.enter_context(tc.tile_pool(name="pstr", bufs=2, space="PSUM"))

        # ---- resident loads ----
        k2p = "(k p) c -> p k c"
        wi1_sb = persist.tile([128, 4, 1536], BF16)
        nc.sync.dma_start(out=wi1_sb[:], in_=wi1_d.rearrange(k2p, p=128))
        wh1_sb = persist.tile([128, 4, 1536], BF16)
        nc.sync.dma_start(out=wh1_sb[:], in_=wh1_d.rearrange(k2p, p=128))
        wi2_sb = persist.tile([128, 4, 1536], BF16)
        nc.sync.dma_start(out=wi2_sb[:], in_=wi2_d.rearrange(k2p, p=128))
        wh2_sb = persist.tile([128, 4, 1536], BF16)
        nc.sync.dma_start(out=wh2_sb[:], in_=wh2_d.rearrange(k2p, p=128))
        b1_sb = persist.tile([128, 1536], F32)
        nc.sync.dma_start(out=b1_sb[:], in_=b1_d)
        b2_sb = persist.tile([1, 1536], BF16)
        nc.sync.dma_start(out=b2_sb[:], in_=b2_d)
        eyestk_sb = persist.tile([128, 32], BF16)
        nc.sync.dma_start(out=eyestk_sb[:], in_=eyestk_d)
        eye128_sb = persist.tile([128, 128], F32)
        nc.sync.dma_start(out=eye128_sb[:], in_=eye128_d)
        ones32_sb = persist.tile([1, 32], BF16)
        nc.sync.dma_start(out=ones32_sb[:], in_=ones32_d)
        ones128b_sb = persist.tile([1, 128], BF16)
        nc.sync.dma_start(out=ones128b_sb[:], in_=ones128b_d)
        ones128r_sb = persist.tile([1, 128], F32R)
        nc.sync.dma_start(out=ones128r_sb[:], in_=ones128r_d)
        bout_sb = persist.tile([128, VL], F32)
        nc.sync.dma_start(out=bout_sb[:], in_=bout_d)
        xg1_sb = persist.tile([128, 16, 1536], BF16)
        outsT = persist.tile([128, 4, T], F32R)

        h_cur = [h_pool.tile([128, 128], F32, name=f"h0_{l}") for l in range(L)]
        hT_cur = [ht_pool.tile([128, 128], BF16, name=f"ht0_{l}") for l in range(L)]
        for l in range(L):
            nc.sync.dma_start(out=h_cur[l][:], in_=h0p_d[l])
            nc.sync.dma_start(out=hT_cur[l][:], in_=h0t_d[l])

        # ---- phase A: xg1 = x @ Wi1 + b1 (bf16 GEMM, fp32 psum) ----
        xt_r = xt_d.rearrange("(k p) m -> p k m", p=128)
        for tt in range(16):
            xa = xt_pool.tile([128, 4, 128], BF16, tag="xa")
            nc.sync.dma_start(out=xa[:], in_=xt_r[:, :, 128 * tt:128 * tt + 128])
            for c3 in range(3):
                cs = slice(512 * c3, 512 * c3 + 512)
                ps = ps_big.tile([128, 512], F32, tag="big", name=f"xga{tt}_{c3}")
                for kc in range(4):
                    nc.tensor.matmul(ps[:], xa[:, kc, :], wi1_sb[:, kc, cs],
                                     start=(kc == 0), stop=(kc == 3))
                # add bias (broadcast tile) while evicting PSUM, cast to bf16
                nc.vector.tensor_add(xg1_sb[:, tt, cs], ps[:], b1_sb[:, cs])

        # ---- recurrence ----
        # PSUM layout per col-group j (cols): [0:128]=hh, [128:256]=r,
        # [256:384]=z, [384:512]=xh.  wh weights are permuted (hh,r,z) so one
        # 384-wide moving slice covers psum [0:384]; wi/xg/bias are permuted
        # (r,z,h) covering psum [128:512].  Matmuls are emitted j-inner
        # ("waves") so consecutive instructions hit different col-groups and
        # stream concurrently through separate XBUSes.
        def rec_step(l, t, h_prev, hT_prev, hT1_now):
            psg = ps_rec.tile([128, PW], F32, tag="rec", name=f"psg{l}_{t}")
            wh_sb = wh1_sb if l == 0 else wh2_sb
            GS = [slice(GW * j, GW * j + 384) for j in range(NJ)]

            def mm(j, pcols, stat, mov, **kw):
                nc.tensor.matmul(psg[32 * j:32 * j + 32, pcols], stat, mov,
                                 skip_group_check=True, **kw)

            if l == 0:
                rb = 32 * (t % 4)
                for j in range(NJ):  # xg wave: psum [128:512] start
                    mm(j, slice(128, 512), eyestk_sb[rb:rb + 32, :],
                       xg1_sb[rb:rb + 32, t // 4, GS[j]],
                       start=True, stop=False, tile_position=(rb, 32 * j))
            else:
                for j in range(NJ):  # bias wave: psum [128:512] start
                    mm(j, slice(128, 512), ones32_sb[:], b2_sb[0:1, GS[j]],
                       start=True, stop=False, tile_position=(0, 32 * j))
                for kc in range(4):  # wi2 waves over hT1(t)
                    stat = hT1_now[:, 32 * kc:32 * kc + 32]
                    for j in range(NJ):
                        mm(j, slice(128, 512), stat, wi2_sb[:, kc, GS[j]],
                           start=False, stop=False, tile_position=(0, 32 * j))
            # recurrent waves; kc=0 split so hh cols [0:128] get start=True
            st0 = hT_prev[:, 0:32]
            for j in range(NJ):
                mm(j, slice(0, 128), st0, wh_sb[:, 0, GW * j:GW * j + 128],
                   start=True, stop=False, tile_position=(0, 32 * j))
            for j in range(NJ):
                mm(j, slice(128, 384), st0,
                   wh_sb[:, 0, GW * j + 128:GW * j + 384],
                   start=False, stop=False, tile_position=(0, 32 * j))
            for kc in range(1, 4):
                stat = hT_prev[:, 32 * kc:32 * kc + 32]
                for j in range(NJ):
                    mm(j, slice(0, 384), stat, wh_sb[:, kc, GS[j]],
                       start=False, stop=(kc == 3), tile_position=(0, 32 * j))
            srz = srz_pool.tile([128, 256], F32, tag="srz", name=f"srz{l}_{t}")
            nc.scalar.activation(srz[:], psg[:, 128:384], SIG)
            hp1 = tmp_pool.tile([128, 128], F32, tag="tmp", name=f"hp1_{l}_{t}")
            nc.vector.tensor_mul(hp1[:], srz[:, 0:128], psg[:, 0:128])
            hp2 = tmp_pool.tile([128, 128], F32, tag="tmp", name=f"hp2_{l}_{t}")
            nc.vector.tensor_add(hp2[:], hp1[:], psg[:, 384:512])
            hc = tmp_pool.tile([128, 128], F32, tag="tmp", name=f"hc_{l}_{t}")
            nc.scalar.activation(hc[:], hp2[:], TANH)
            dd = tmp_pool.tile([128, 128], F32, tag="tmp", name=f"dd_{l}_{t}")
            nc.vector.tensor_sub(dd[:], h_prev[:], hc[:])
            ee = tmp_pool.tile([128, 128], F32, tag="tmp", name=f"ee_{l}_{t}")
            nc.vector.tensor_mul(ee[:], srz[:, 128:256], dd[:])
            h_new = h_pool.tile([128, 128], F32, tag="h", name=f"h_{l}_{t}")
            nc.vector.tensor_add(h_new[:], ee[:], hc[:])
            trp = ps_tr.tile([128, 128], F32, tag="tr", name=f"tr{l}_{t}")
            nc.tensor.transpose(trp[:], h_new[:], eye128_sb[:])
            hT_new = ht_pool.tile([128, 128], BF16, tag="ht", name=f"ht_{l}_{t}")
            nc.scalar.activation(hT_new[:], trp[:], COPY)
            if l == 1:
                for kc in range(4):
                    nc.scalar.activation(outsT[:, kc, 32 * t:32 * t + 32],
                                         trp[:, 32 * kc:32 * kc + 32], COPY)
            return h_new, hT_new

        hT1_at = {}
        for t in range(S):
            h_cur[0], hT_cur[0] = rec_step(0, t, h_cur[0], hT_cur[0], None)
            hT1_at[t] = hT_cur[0]
            if t >= 1:
                h_cur[1], hT_cur[1] = rec_step(1, t - 1, h_cur[1], hT_cur[1],
                                               hT1_at.pop(t - 1))
        h_cur[1], hT_cur[1] = rec_step(1, S - 1, h_cur[1], hT_cur[1],
                                       hT1_at.pop(S - 1))

        for l in range(L):
            nc.sync.dma_start(out=hfin_d[l], in_=h_cur[l][:])

        # ---- output GEMM: logits = outs @ Wout_local.T + bout_local ----
        for vc in range(NVC):
            vs = slice(VC * vc, VC * vc + VC)
            wts = []
            for kc in range(4):
                wt = wout_pool.tile([128, VC], F32R, tag="wo", name=f"wo{vc}_{kc}")
                nc.sync.dma_start(out=wt[:], in_=wout_d[128 * kc:128 * kc + 128, vs])
                wts.append(wt)
            for tt in range(16):
                ps = ps_big.tile([128, VC], F32, tag="big", name=f"og{vc}_{tt}")
                for kc in range(4):
                    nc.tensor.matmul(ps[:],
                                     outsT[:, kc, 128 * tt:128 * tt + 128],
                                     wts[kc][:], start=(kc == 0), stop=(kc == 3))
                lsb = lg_pool.tile([128, VC], F32, tag="lg", name=f"lg{vc}_{tt}")
                # bias add (broadcast tile) while evicting PSUM
                nc.vector.tensor_add(lsb[:], ps[:], bout_sb[:, vs])
                nc.sync.dma_start(out=logits_d[128 * tt:128 * tt + 128, vs],
                                  in_=lsb[:])

    nc.compile()
    return nc


_NC = None


def _get_nc():
    global _NC
    if _NC is None:
        _NC = _build()
    return _NC


def _prep_in_maps(inputs, hidden, emb_table, Wir, bir, Wiz, biz, Wih, bih,
                  Whr, Whz, Whh, Wout, bout):
    perm_i = _perm1536((0, 1, 2))   # wi/xg/bias groups: [r|z|h]
    perm_h = _perm1536((2, 0, 1))   # wh groups: [hh|r|z]
    inputs = np.asarray(inputs)
    x = np.asarray(emb_table, np.float32)[inputs.reshape(-1)]   # [T, E]
    xt = np.ascontiguousarray(x.T).astype(BF16NP)               # [E, T]

    def wcat(Wr, Wz, Wh, l, perm):
        w = np.concatenate([np.asarray(Wr[l], np.float32).T,
                            np.asarray(Wz[l], np.float32).T,
                            np.asarray(Wh[l], np.float32).T], axis=1)
        return np.ascontiguousarray(w[:, perm]).astype(BF16NP)

    def bcat(br, bz, bh, l, perm):
        b = np.concatenate([np.asarray(br[l], np.float32),
                            np.asarray(bz[l], np.float32),
                            np.asarray(bh[l], np.float32)])
        return np.ascontiguousarray(b[perm])

    wi1 = wcat(Wir, Wiz, Wih, 0, perm_i)
    wh1 = wcat(Whr, Whz, Whh, 0, perm_h)
    wi2 = wcat(Wir, Wiz, Wih, 1, perm_i)
    wh2 = wcat(Whr, Whz, Whh, 1, perm_h)
    # layer-1 input bias, broadcast along partitions for the DVE add
    b1 = np.broadcast_to(bcat(bir, biz, bih, 0, perm_i)[None, :],
                         (128, 1536)).astype(np.float32).copy()
    # layer-2 bias enters through the per-step bias wave (bf16 row)
    b2 = bcat(bir, biz, bih, 1, perm_i)[None, :].astype(BF16NP)

    hid = np.asarray(hidden, np.float32)                        # [2, B, H]
    h0p = hid.reshape(L, B, 4, 128).transpose(0, 2, 1, 3).reshape(L, 128, 128)
    h0p = np.ascontiguousarray(h0p)
    h0t = hid.reshape(L, B, 4, 128).transpose(0, 3, 2, 1).reshape(L, 128, 128)
    h0t = np.ascontiguousarray(h0t).astype(BF16NP)

    eyestk = np.tile(np.eye(32, dtype=np.float32), (4, 1)).astype(BF16NP)
    eye128 = np.eye(128, dtype=np.float32)
    ones32 = np.ones((1, 32), BF16NP)
    ones128b = np.ones((1, 128), BF16NP)
    ones128r = np.ones((1, 128), np.float32)

    woutT = np.ascontiguousarray(np.asarray(Wout, np.float32).T)  # [H, V]
    boutf = np.asarray(bout, np.float32)

    base = dict(xt=xt, wi1=wi1, wh1=wh1, wi2=wi2, wh2=wh2, b1=b1, b2=b2,
                h0p=h0p, h0t=h0t, eyestk=eyestk, eye128=eye128,
                ones32=ones32, ones128b=ones128b, ones128r=ones128r)
    in_maps = []
    for c in range(NCORES):
        m = dict(base)
        m["wout"] = np.ascontiguousarray(woutT[:, c * VL:(c + 1) * VL])
        m["bout"] = np.broadcast_to(boutf[c * VL:(c + 1) * VL][None, :],
                                    (128, VL)).astype(np.float32).copy()
        in_maps.append(m)
    return in_maps


def _assemble(results):
    logits = np.concatenate(
        [results[c]["logits"].reshape(S, B, VL) for c in range(NCORES)], axis=2)
    hf = results[0]["hfin"]                                      # [2,128,128]
    hstk = hf.reshape(L, 4, 32, 128).transpose(0, 2, 1, 3).reshape(L, B, H)
    return logits.astype(np.float32), np.ascontiguousarray(hstk, np.float32)


def run_on_hw(in_maps, trace=False, **kw):
    nc = _get_nc()
    return run_bass_kernel_spmd(nc, in_maps, core_ids=list(range(NCORES)),
                                trace=trace, **kw)


def kernel(**inputs):
    in_maps = _prep_in_maps(**inputs)
    res = run_on_hw(in_maps)
    return _assemble(res.results)


# revision 18
# speedup vs baseline: 1.1492x; 1.1492x over previous
"""Trainium2 Bass kernel for a 2-layer GRU LM step (nn_GRU_83519934038617).

Model (fp32 reference):
  x = emb[inputs]                                  [S=64, B=32, E=512]
  2 GRU layers (H=512), logits = outs @ Wout.T + bout   [S, B, V=32000]
  returns (logits, stacked final hidden [2, B, H])

Strategy (8 cores, no collectives):
  - GRU recurrence replicated on every core (it is weight-streaming bound,
    so batch-sharding would not make it faster); vocab dim of the output
    projection sharded 8 ways (V_local = 4000) since logits dominate memory.
  - Per step, gates are computed with 4 concurrent column-group matmuls
    (tile_position col tiling, bf16) so the gate pre-activations land in
    PSUM as [128 partitions = (chunk j, batch b), 512 = r|z|hh|xh], which
    makes all elementwise work full-width [128, *] ops.
  - x-projections (xg1) are batched into one big GEMM up front; their
    per-step contribution enters the gate PSUM through an identity-
    stationary matmul (no DVE adds). Layer-2's input projection is folded
    into the per-step matmul as extra contraction tiles over hT1(t).
  - h is kept in two layouts: [(j,b), h'] for elementwise, and a bf16
    transpose [h', (kc,b)] (produced by one PE transpose per step) that
    feeds the next step's stationaries and accumulates into outsT for the
    final fp32r output GEMM.
"""
import numpy as np
import ml_dtypes
from contextlib import ExitStack

import concourse.tile as tile
from concourse import bacc, mybir
from concourse.bass_utils import run_bass_kernel_spmd

F32 = mybir.dt.float32
F32R = mybir.dt.float32r
BF16 = mybir.dt.bfloat16
BF16NP = ml_dtypes.bfloat16

S, B, E, H, V, L = 64, 32, 512, 512, 32000, 2
NCORES = 8
VL = V // NCORES          # 4000
T = S * B                 # 2048 tokens
NJ = 4                    # column groups (h-chunks of 128)
GW = 384                  # r|z|h group width in the 1536 weight layout
PW = 512                  # psum width per step: r|z|hh|xh
VC = 500                  # vocab chunk for the output GEMM (8 chunks)
NVC = VL // VC            # 8

SIG = mybir.ActivationFunctionType.Sigmoid
TANH = mybir.ActivationFunctionType.Tanh
COPY = mybir.ActivationFunctionType.Copy


def _perm1536(order):
    # new col j*384 + s*128 + hp  <-  old col order[s]*512 + 128*j + hp
    # order maps section s (within a 384-wide group) to gate index
    # (0=r, 1=z, 2=h in the concatenated [r|z|h] layout).
    perm = np.empty(1536, np.int64)
    for j in range(4):
        for s, g in enumerate(order):
            for hp in range(128):
                perm[j * 384 + s * 128 + hp] = g * 512 + 128 * j + hp
    return perm


def _build():
    nc = bacc.Bacc("TRN2", target_bir_lowering=False, debug=False,
                   num_devices=NCORES)

    def din(name, shape, dt):
        return nc.dram_tensor(name, shape, dt, kind="ExternalInput").ap()

    xt_d = din("xt", [E, T], BF16)
    wi1_d = din("wi1", [E, 1536], BF16)
    wh1_d = din("wh1", [H, 1536], BF16)
    wi2_d = din("wi2", [H, 1536], BF16)
    wh2_d = din("wh2", [H, 1536], BF16)
    b1_d = din("b1", [128, 1536], F32)   # broadcast along partitions (DVE add)
    b2_d = din("b2", [1, 1536], BF16)
    h0p_d = din("h0p", [L, 128, 128], F32)
    h0t_d = din("h0t", [L, 128, 128], BF16)
    eyestk_d = din("eyestk", [128, 32], BF16)
    eye128_d = din("eye128", [128, 128], F32)
    ones32_d = din("ones32", [1, 32], BF16)
    ones128b_d = din("ones128b", [1, 128], BF16)
    ones128r_d = din("ones128r", [1, 128], F32R)
    wout_d = din("wout", [H, VL], F32R)
    bout_d = din("bout", [128, VL], F32)  # broadcast along partitions (DVE add)

    logits_d = nc.dram_tensor("logits", [T, VL], F32, kind="ExternalOutput").ap()
    hfin_d = nc.dram_tensor("hfin", [L, 128, 128], F32, kind="ExternalOutput").ap()

    with tile.TileContext(nc) as tc, ExitStack() as ctx:
        persist = ctx.enter_context(tc.tile_pool(name="persist", bufs=1))
        xt_pool = ctx.enter_context(tc.tile_pool(name="xt", bufs=2))
        wout_pool = ctx.enter_context(tc.tile_pool(name="wout", bufs=8))
        lg_pool = ctx.enter_context(tc.tile_pool(name="lg", bufs=3))
        srz_pool = ctx.enter_context(tc.tile_pool(name="srz", bufs=4))
        tmp_pool = ctx.enter_context(tc.tile_pool(name="tmp", bufs=8))
        h_pool = ctx.enter_context(tc.tile_pool(name="hp", bufs=6))
        ht_pool = ctx.enter_context(tc.tile_pool(name="htp", bufs=6))
        ps_big = ctx.enter_context(tc.tile_pool(name="psbig", bufs=2, space="PSUM"))
        ps_rec = ctx.enter_context(tc.tile_pool(name="psrec", bufs=4, space="PSUM"))
        ps_tr = ctx.enter_context(tc.tile_pool(name="pstr", bufs=2, space="PSUM"))

        # ---- resident loads ----
        k2p = "(k p) c -> p k c"
        wi1_sb = persist.tile([128, 4, 1536], BF16)
        nc.sync.dma_start(out=wi1_sb[:], in_=wi1_d.rearrange(k2p, p=128))
        wh1_sb = persist.tile([128, 4, 1536], BF16)
        nc.sync.dma_start(out=wh1_sb[:], in_=wh1_d.rearrange(k2p, p=128))
        wi2_sb = persist.tile([128, 4, 1536], BF16)
        nc.sync.dma_start(out=wi2_sb[:], in_=wi2_d.rearrange(k2p, p=128))
        wh2_sb = persist.tile([128, 4, 1536], BF16)
        nc.sync.dma_start(out=wh2_sb[:], in_=wh2_d.rearrange(k2p, p=128))
        b1_sb = persist.tile([128, 1536], F32)
        nc.sync.dma_start(out=b1_sb[:], in_=b1_d)
        b2_sb = persist.tile([1, 1536], BF16)
        nc.sync.dma_start(out=b2_sb[:], in_=b2_d)
        eyestk_sb = persist.tile([128, 32], BF16)
        nc.sync.dma_start(out=eyestk_sb[:], in_=eyestk_d)
        eye128_sb = persist.tile([128, 128], F32)
        nc.sync.dma_start(out=eye128_sb[:], in_=eye128_d)
        ones32_sb = persist.tile([1, 32], BF16)
        nc.sync.dma_start(out=ones32_sb[:], in_=ones32_d)
        ones128b_sb = persist.tile([1, 128], BF16)
        nc.sync.dma_start(out=ones128b_sb[:], in_=ones128b_d)
        ones128r_sb = persist.tile([1, 128], F32R)
        nc.sync.dma_start(out=ones128r_sb[:], in_=ones128r_d)
        bout_sb = persist.tile([128, VL], F32)
        nc.sync.dma_start(out=bout_sb[:], in_=bout_d)
        xg1_sb = persist.tile([128, 16, 1536], BF16)
        outsT = persist.tile([128, 4, T], F32R)

        h_cur = [h_pool.tile([128, 128], F32, name=f"h0_{l}") for l in range(L)]
        hT_cur = [ht_pool.tile([128, 128], BF16, name=f"ht0_{l}") for l in range(L)]
        for l in range(L):
            nc.sync.dma_start(out=h_cur[l][:], in_=h0p_d[l])
            nc.sync.dma_start(out=hT_cur[l][:], in_=h0t_d[l])

        # ---- phase A: xg1 = x @ Wi1 + b1 (bf16 GEMM, fp32 psum) ----
        xt_r = xt_d.rearrange("(k p) m -> p k m", p=128)
        for tt in range(16):
            xa = xt_pool.tile([128, 4, 128], BF16, tag="xa")
            nc.sync.dma_start(out=xa[:], in_=xt_r[:, :, 128 * tt:128 * tt + 128])
            for c3 in range(3):
                cs = slice(512 * c3, 512 * c3 + 512)
                ps = ps_big.tile([128, 512], F32, tag="big", name=f"xga{tt}_{c3}")
                for kc in range(4):
                    nc.tensor.matmul(ps[:], xa[:, kc, :], wi1_sb[:, kc, cs],
                                     start=(kc == 0), stop=(kc == 3))
                # add bias (broadcast tile) while evicting PSUM, cast to bf16
                nc.vector.tensor_add(xg1_sb[:, tt, cs], ps[:], b1_sb[:, cs])

        # ---- recurrence ----
        # PSUM layout per col-group j (cols): [0:128]=hh, [128:256]=r,
        # [256:384]=z, [384:512]=xh.  wh weights are permuted (hh,r,z) so one
        # 384-wide moving slice covers psum [0:384]; wi/xg/bias are permuted
        # (r,z,h) covering psum [128:512].  Matmuls are emitted j-inner
        # ("waves") so consecutive instructions hit different col-groups and
        # stream concurrently through separate XBUSes.
        def rec_mm(l, t, hT_prev, hT1_now):
            psg = ps_rec.tile([128, PW], F32, tag="rec", name=f"psg{l}_{t}")
            wh_sb = wh1_sb if l == 0 else wh2_sb
            GS = [slice(GW * j, GW * j + 384) for j in range(NJ)]

            def mm(j, pcols, stat, mov, **kw):
                nc.tensor.matmul(psg[32 * j:32 * j + 32, pcols], stat, mov,
                                 skip_group_check=True, **kw)

            if l == 0:
                rb = 32 * (t % 4)
                for j in range(NJ):  # xg wave: psum [128:512] start
                    mm(j, slice(128, 512), eyestk_sb[rb:rb + 32, :],
                       xg1_sb[rb:rb + 32, t // 4, GS[j]],
                       start=True, stop=False, tile_position=(rb, 32 * j))
            else:
                for j in range(NJ):  # bias wave: psum [128:512] start
                    mm(j, slice(128, 512), ones32_sb[:], b2_sb[0:1, GS[j]],
                       start=True, stop=False, tile_position=(0, 32 * j))
                for kc in range(4):  # wi2 waves over hT1(t)
                    stat = hT1_now[:, 32 * kc:32 * kc + 32]
                    for j in range(NJ):
                        mm(j, slice(128, 512), stat, wi2_sb[:, kc, GS[j]],
                           start=False, stop=False, tile_position=(0, 32 * j))
            # recurrent waves; kc=0 split so hh cols [0:128] get start=True
            st0 = hT_prev[:, 0:32]
            for j in range(NJ):
                mm(j, slice(0, 128), st0, wh_sb[:, 0, GW * j:GW * j + 128],
                   start=True, stop=False, tile_position=(0, 32 * j))
            for j in range(NJ):
                mm(j, slice(128, 384), st0,
                   wh_sb[:, 0, GW * j + 128:GW * j + 384],
                   start=False, stop=False, tile_position=(0, 32 * j))
            for kc in range(1, 4):
                stat = hT_prev[:, 32 * kc:32 * kc + 32]
                for j in range(NJ):
                    mm(j, slice(0, 384), stat, wh_sb[:, kc, GS[j]],
                       start=False, stop=(kc == 3), tile_position=(0, 32 * j))
            return psg

        def rec_ve(l, t, psg, h_prev):
            srz = srz_pool.tile([128, 256], F32, tag="srz", name=f"srz{l}_{t}")
            nc.scalar.activation(srz[:], psg[:, 128:384], SIG)
            hp1 = tmp_pool.tile([128, 128], F32, tag="tmp", name=f"hp1_{l}_{t}")
            nc.vector.tensor_mul(hp1[:], srz[:, 0:128], psg[:, 0:128])
            hp2 = tmp_pool.tile([128, 128], F32, tag="tmp", name=f"hp2_{l}_{t}")
            nc.vector.tensor_add(hp2[:], hp1[:], psg[:, 384:512])
            hc = tmp_pool.tile([128, 128], F32, tag="tmp", name=f"hc_{l}_{t}")
            nc.scalar.activation(hc[:], hp2[:], TANH)
            dd = tmp_pool.tile([128, 128], F32, tag="tmp", name=f"dd_{l}_{t}")
            nc.vector.tensor_sub(dd[:], h_prev[:], hc[:])
            ee = tmp_pool.tile([128, 128], F32, tag="tmp", name=f"ee_{l}_{t}")
            nc.vector.tensor_mul(ee[:], srz[:, 128:256], dd[:])
            h_new = h_pool.tile([128, 128], F32, tag="h", name=f"h_{l}_{t}")
            nc.vector.tensor_add(h_new[:], ee[:], hc[:])
            trp = ps_tr.tile([128, 128], F32, tag="tr", name=f"tr{l}_{t}")
            nc.tensor.transpose(trp[:], h_new[:], eye128_sb[:])
            hT_new = ht_pool.tile([128, 128], BF16, tag="ht", name=f"ht_{l}_{t}")
            nc.scalar.activation(hT_new[:], trp[:], COPY)
            if l == 1:
                for kc in range(4):
                    nc.scalar.activation(outsT[:, kc, 32 * t:32 * t + 32],
                                         trp[:, 32 * kc:32 * kc + 32], COPY)
            return h_new, hT_new

        # Emission order per pair: both layers' MM waves first, then both
        # elementwise chains — so PE never head-of-line blocks on a chain.
        hT1_at = {}
        for t in range(S):
            ps1 = rec_mm(0, t, hT_cur[0], None)
            ps2 = rec_mm(1, t - 1, hT_cur[1], hT1_at.pop(t - 1)) if t >= 1 else None
            h_cur[0], hT_cur[0] = rec_ve(0, t, ps1, h_cur[0])
            hT1_at[t] = hT_cur[0]
            if ps2 is not None:
                h_cur[1], hT_cur[1] = rec_ve(1, t - 1, ps2, h_cur[1])
        ps2 = rec_mm(1, S - 1, hT_cur[1], hT1_at.pop(S - 1))
        h_cur[1], hT_cur[1] = rec_ve(1, S - 1, ps2, h_cur[1])

        for l in range(L):
            nc.sync.dma_start(out=hfin_d[l], in_=h_cur[l][:])

        # ---- output GEMM: logits = outs @ Wout_local.T + bout_local ----
        for vc in range(NVC):
            vs = slice(VC * vc, VC * vc + VC)
            wts = []
            for kc in range(4):
                wt = wout_pool.tile([128, VC], F32R, tag="wo", name=f"wo{vc}_{kc}")
                nc.sync.dma_start(out=wt[:], in_=wout_d[128 * kc:128 * kc + 128, vs])
                wts.append(wt)
            for tt in range(16):
                ps = ps_big.tile([128, VC], F32, tag="big", name=f"og{vc}_{tt}")
                for kc in range(4):
                    nc.tensor.matmul(ps[:],
                                     outsT[:, kc, 128 * tt:128 * tt + 128],
                                     wts[kc][:], start=(kc == 0), stop=(kc == 3))
                lsb = lg_pool.tile([128, VC], F32, tag="lg", name=f"lg{vc}_{tt}")
                # bias add (broadcast tile) while evicting PSUM
                nc.vector.tensor_add(lsb[:], ps[:], bout_sb[:, vs])
                nc.sync.dma_start(out=logits_d[128 * tt:128 * tt + 128, vs],
                                  in_=lsb[:])

    nc.compile()
    return nc


_NC = None


def _get_nc():
    global _NC
    if _NC is None:
        _NC = _build()
    return _NC


def _prep_in_maps(inputs, hidden, emb_table, Wir, bir, Wiz, biz, Wih, bih,
                  Whr, Whz, Whh, Wout, bout):
    perm_i = _perm1536((0, 1, 2))   # wi/xg/bias groups: [r|z|h]
    perm_h = _perm1536((2, 0, 1))   # wh groups: [hh|r|z]
    inputs = np.asarray(inputs)
    x = np.asarray(emb_table, np.float32)[inputs.reshape(-1)]   # [T, E]
    xt = np.ascontiguousarray(x.T).astype(BF16NP)               # [E, T]

    def wcat(Wr, Wz, Wh, l, perm):
        w = np.concatenate([np.asarray(Wr[l], np.float32).T,
                            np.asarray(Wz[l], np.float32).T,
                            np.asarray(Wh[l], np.float32).T], axis=1)
        return np.ascontiguousarray(w[:, perm]).astype(BF16NP)

    def bcat(br, bz, bh, l, perm):
        b = np.concatenate([np.asarray(br[l], np.float32),
                            np.asarray(bz[l], np.float32),
                            np.asarray(bh[l], np.float32)])
        return np.ascontiguousarray(b[perm])

    wi1 = wcat(Wir, Wiz, Wih, 0, perm_i)
    wh1 = wcat(Whr, Whz, Whh, 0, perm_h)
    wi2 = wcat(Wir, Wiz, Wih, 1, perm_i)
    wh2 = wcat(Whr, Whz, Whh, 1, perm_h)
    # layer-1 input bias, broadcast along partitions for the DVE add
    b1 = np.broadcast_to(bcat(bir, biz, bih, 0, perm_i)[None, :],
                         (128, 1536)).astype(np.float32).copy()
    # layer-2 bias enters through the per-step bias wave (bf16 row)
    b2 = bcat(bir, biz, bih, 1, perm_i)[None, :].astype(BF16NP)

    hid = np.asarray(hidden, np.float32)                        # [2, B, H]
    h0p = hid.reshape(L, B, 4, 128).transpose(0, 2, 1, 3).reshape(L, 128, 128)
    h0p = np.ascontiguousarray(h0p)
    h0t = hid.reshape(L, B, 4, 128).transpose(0, 3, 2, 1).reshape(L, 128, 128)
    h0t = np.ascontiguousarray(h0t).astype(BF16NP)

    eyestk = np.tile(np.eye(32, dtype=np.float32), (4, 1)).astype(BF16NP)
    eye128 = np.eye(128, dtype=np.float32)
    ones32 = np.ones((1, 32), BF16NP)
    ones128b = np.ones((1, 128), BF16NP)
    ones128r = np.ones((1, 128), np.float32)

    woutT = np.ascontiguousarray(np.asarray(Wout, np.float32).T)  # [H, V]
    boutf = np.asarray(bout, np.float32)

    base = dict(xt=xt, wi1=wi1, wh1=wh1, wi2=wi2, wh2=wh2, b1=b1, b2=b2,
                h0p=h0p, h0t=h0t, eyestk=eyestk, eye128=eye128,
                ones32=ones32, ones128b=ones128b, ones128r=ones128r)
    in_maps = []
    for c in range(NCORES):
        m = dict(base)
        m["wout"] = np.ascontiguousarray(woutT[:, c * VL:(c + 1) * VL])
        m["bout"] = np.broadcast_to(boutf[c * VL:(c + 1) * VL][None, :],
                                    (128, VL)).astype(np.float32).copy()
        in_maps.append(m)
    return in_maps


def _assemble(results):
    logits = np.concatenate(
        [results[c]["logits"].reshape(S, B, VL) for c in range(NCORES)], axis=2)
    hf = results[0]["hfin"]                                      # [2,128,128]
    hstk = hf.reshape(L, 4, 32, 128).transpose(0, 2, 1, 3).reshape(L, B, H)
    return logits.astype(np.float32), np.ascontiguousarray(hstk, np.float32)


def run_on_hw(in_maps, trace=False, **kw):
    nc = _get_nc()
    return run_bass_kernel_spmd(nc, in_maps, core_ids=list(range(NCORES)),
                                trace=trace, **kw)


def kernel(**inputs):
    in_maps = _prep_in_maps(**inputs)
    res = run_on_hw(in_maps)
    return _assemble(res.results)
